# revision 1
# baseline (speedup 1.0000x reference)
"""GATv2 message-passing kernel for 8 trn2 NeuronCores (Bass/Tile).

Strategy (dst-partitioned, per the graph/data-parallel hint):
  * nodes are partitioned across the 8 cores (6250 dst nodes each); each
    core owns all edges whose destination falls in its partition
    (self-loops included), so the segment softmax and weighted scatter
    are purely local (nothing to all-reduce).
  * source-node features are a full "halo": every core computes the
    xl' projection table for all 50k nodes on device and gathers the
    rows its edges need with indirect DMA (OOB-skipped padding slots).
  * weight matrices are replicated; |att| is folded into Wl/Wr on the
    host and the H axis is permuted so positive-sign att components are
    contiguous, turning the per-edge attention dot product into two
    free-axis reduction slices of leaky_relu(xl'[src] + xr'[dst]).
  * per dst-block (128 rows x K slots, degree-sorted so K is tight):
    e = sum_pos - sum_neg - pad_mask; ex = exp(e) (max-subtraction is
    unnecessary: |e| is bounded); out = (sum_k ex*G) / sum_k ex,
    un-scaled by 1/|att|, + bias, ELU; final logits via PE matmul with
    Wo^T, emitted transposed and unpermuted on the host.
"""
import sys
sys.path.insert(0, '/opt/trn_rl_repo')
import numpy as np
import concourse.bass as bass
import concourse.mybir as mybir
import concourse.tile as tile
from concourse.masks import make_identity
from concourse.vector_clock import ScopedClock

F32 = mybir.dt.float32
I32 = mybir.dt.int32
AX = mybir.AxisListType
OP = mybir.AluOpType
AF = mybir.ActivationFunctionType

NEG_SLOPE = 0.2
SENTINEL = 1 << 20
BIG = 1.0e9
N_CORES = 8


# --------------------------------------------------------------------------
# Workarounds for the toolchain in this container: the walrus build rejects
# instructions carrying more than ~2 semaphore waits, so the Tile epilogue
# drain and any multi-wait instruction get their waits hoisted onto
# dedicated NOPs.
# --------------------------------------------------------------------------

def _patched_drain_and_barrier(self, tick_clock, wait_clock):
    nc = self.nc
    probe = nc.sync.nop(nofuse=True)
    wait_clock.add_sem_waits(probe.ins, ScopedClock({None: tick_clock.global_clock}))
    si = probe.ins.sync_info
    waits = list(si.on_wait) if si is not None else []
    if len(waits) > 1:
        si.on_wait.clear()
        si.on_wait.append(waits[0])
        for w in waits[1:]:
            n = nc.sync.nop(nofuse=True)
            nsi = n.ins.sync_info
            if nsi is None:
                n.ins.sync_info = mybir.SyncInfo(on_wait=[w], on_update=[])
            else:
                nsi.on_wait.append(w)
    nc.sync.drain()
    nc.all_engine_barrier()
    assert self.sems is not None
    popped = nc._tile_sem_poison_stack.pop()
    assert popped is self._sem_poison
    nc.clear_and_free_semaphores(list(self.sems.allocated().values()))
    nc.all_engine_barrier()


tile.TileContext._drain_and_barrier = _patched_drain_and_barrier


def split_excess_waits(nc, limit=1):
    for f in nc.m.functions:
        for bb in f.blocks:
            insts = bb.instructions
            new_insts = []
            bb_changed = False
            for inst in insts:
                si = inst.sync_info
                waits = list(si.on_wait) if (si is not None and si.on_wait) else []
                if len(waits) > limit:
                    bb_changed = True
                    excess = waits[:-limit]
                    keep = waits[-limit:]
                    for i in range(0, len(excess), limit):
                        chunk = excess[i:i + limit]
                        nop = mybir.InstNoOp(
                            name=nc.get_next_instruction_name(),
                            sync_info=mybir.SyncInfo(on_wait=list(chunk), on_update=[]),
                            bass_nofuse=True,
                            engine=inst.engine,
                        )
                        new_insts.append(nop)
                    si.on_wait.clear()
                    for w in keep:
                        si.on_wait.append(w)
                new_insts.append(inst)
            if bb_changed:
                bb.instructions = new_insts


# --------------------------------------------------------------------------
# Host-side preprocessing: sharding, degree-sorted padded CSR, weight prep.
# --------------------------------------------------------------------------

def build_host_data(x_float, x_binary, edge_index, Wl, bl, Wr, br, att,
                    gat_bias, Wo, bo, n_cores=N_CORES):
    N, F = x_float.shape
    H = Wl.shape[0]
    in_dim = Wl.shape[1]
    assert in_dim == F + 1

    att = np.asarray(att, dtype=np.float64)
    pos = np.where(att > 0)[0]
    neg = np.where(att <= 0)[0]
    perm_h = np.concatenate([pos, neg]).astype(np.int64)
    P = len(pos)
    absa = np.abs(att[perm_h])
    absa_safe = np.maximum(absa, 1e-30)

    Wl_p = (absa[:, None] * np.asarray(Wl, np.float64)[perm_h]).astype(np.float32)
    Wr_p = (absa[:, None] * np.asarray(Wr, np.float64)[perm_h]).astype(np.float32)
    bl_p = (absa * np.asarray(bl, np.float64)[perm_h]).astype(np.float32)
    br_p = (absa * np.asarray(br, np.float64)[perm_h]).astype(np.float32)
    invatt = (1.0 / absa_safe).astype(np.float32)
    gb_p = np.asarray(gat_bias)[perm_h].astype(np.float32)
    Wo_p = np.asarray(Wo)[:, perm_h].astype(np.float32)

    xT = np.empty((in_dim, N), dtype=np.float32)
    xT[:F, :] = np.asarray(x_float, dtype=np.float32).T
    xT[F, :] = np.asarray(x_binary, dtype=np.float32)

    src = np.asarray(edge_index[0], dtype=np.int64)
    dst = np.asarray(edge_index[1], dtype=np.int64)
    loops = np.arange(N, dtype=np.int64)
    src = np.concatenate([src, loops])
    dst = np.concatenate([dst, loops])

    n_own = N // n_cores
    n_pad_own = -(-n_own // 128) * 128
    n_blocks = n_pad_own // 128

    order = np.argsort(dst, kind='stable')
    src_s = src[order]
    dst_s = dst[order]
    starts = np.searchsorted(dst_s, np.arange(N))
    deg = np.searchsorted(dst_s, np.arange(N) + 1) - starts

    cores = []
    for c in range(n_cores):
        own = np.arange(c * n_own, (c + 1) * n_own, dtype=np.int64)
        o = np.argsort(deg[own], kind='stable')
        own_sorted = own[o]
        d_sorted = deg[own][o]
        n_fake = n_pad_own - n_own
        nodes = np.concatenate([np.full(n_fake, -1, dtype=np.int64), own_sorted])
        degs = np.concatenate([np.ones(n_fake, dtype=np.int64), d_sorted])
        Ks = [int(degs[b * 128:(b + 1) * 128].max()) for b in range(n_blocks)]
        cores.append(dict(nodes=nodes, degs=degs, Ks=Ks))

    Kb = [max(cores[c]['Ks'][b] for c in range(n_cores)) for b in range(n_blocks)]
    W = int(np.sum(Kb))
    col_off = np.concatenate([[0], np.cumsum(Kb)]).astype(np.int64)

    per_core = []
    for c in range(n_cores):
        nodes = cores[c]['nodes']
        degs = cores[c]['degs']
        idxg = np.full((128, W), SENTINEL, dtype=np.int32)
        maskb = np.full((128, W), BIG, dtype=np.float32)
        for b in range(n_blocks):
            cs = int(col_off[b])
            for r in range(128):
                node = nodes[b * 128 + r]
                if node < 0:
                    idxg[r, cs] = 0
                    maskb[r, cs] = 0.0
                    continue
                d = int(degs[b * 128 + r])
                e0 = int(starts[node])
                idxg[r, cs:cs + d] = src_s[e0:e0 + d].astype(np.int32)
                maskb[r, cs:cs + d] = 0.0

        xT_own = np.zeros((in_dim, n_pad_own), dtype=np.float32)
        valid = nodes >= 0
        xT_own[:, valid] = xT[:, nodes[valid]]

        per_core.append({
            "xT": xT,
            "xT_own": xT_own,
            "idxg": idxg,
            "maskb": maskb,
            "WlT": np.ascontiguousarray(Wl_p.T),
            "WrT": np.ascontiguousarray(Wr_p.T),
            "blrep": np.tile(bl_p[None, :], (128, 1)),
            "brrep": np.tile(br_p[None, :], (128, 1)),
            "invrep": np.tile(invatt[None, :], (128, 1)),
            "gbrep": np.tile(gb_p[None, :], (128, 1)),
            "WoT": np.ascontiguousarray(Wo_p.T),
            "bo_col": np.asarray(bo, dtype=np.float32)[:, None],
        })

    meta = dict(N=N, F=F, H=H, in_dim=in_dim, P=P, n_own=n_own,
                n_pad_own=n_pad_own, n_blocks=n_blocks, Kb=Kb, W=W,
                col_off=col_off, n_cores=n_cores,
                core_nodes=[cores[c]['nodes'] for c in range(n_cores)])
    return meta, per_core


# --------------------------------------------------------------------------
# Device program (one SPMD NEFF for all 8 cores).
# --------------------------------------------------------------------------

def build_device_program(meta):
    N = meta['N']; F = meta['F']; H = meta['H']; in_dim = meta['in_dim']
    P = meta['P']; n_pad_own = meta['n_pad_own']; n_blocks = meta['n_blocks']
    Kb = meta['Kb']; W = meta['W']; col_off = meta['col_off']

    n_tab_tiles = -(-N // 128)
    N_tab = n_tab_tiles * 128
    Kmax = max(Kb)

    nc = bass.Bass(target_bir_lowering=False)
    xT = nc.dram_tensor("xT", [in_dim, N], F32, kind="ExternalInput")
    xT_own = nc.dram_tensor("xT_own", [in_dim, n_pad_own], F32, kind="ExternalInput")
    idxg = nc.dram_tensor("idxg", [128, W], I32, kind="ExternalInput")
    maskb = nc.dram_tensor("maskb", [128, W], F32, kind="ExternalInput")
    WlT = nc.dram_tensor("WlT", [in_dim, H], F32, kind="ExternalInput")
    WrT = nc.dram_tensor("WrT", [in_dim, H], F32, kind="ExternalInput")
    blrep = nc.dram_tensor("blrep", [128, H], F32, kind="ExternalInput")
    brrep = nc.dram_tensor("brrep", [128, H], F32, kind="ExternalInput")
    invrep = nc.dram_tensor("invrep", [128, H], F32, kind="ExternalInput")
    gbrep = nc.dram_tensor("gbrep", [128, H], F32, kind="ExternalInput")
    WoT = nc.dram_tensor("WoT", [H, F], F32, kind="ExternalInput")
    bo_col = nc.dram_tensor("bo_col", [F, 1], F32, kind="ExternalInput")
    outT = nc.dram_tensor("outT", [F, n_pad_own], F32, kind="ExternalOutput")

    with tile.TileContext(nc) as tc:
        with (
            tc.tile_pool(name="const", bufs=1) as cpool,
            tc.tile_pool(name="dram", bufs=1, space="DRAM") as dpool,
            tc.tile_pool(name="xstage", bufs=2) as xpool,
            tc.tile_pool(name="proj", bufs=3) as ppool,
            tc.tile_pool(name="gat", bufs=3) as gpool,
            tc.tile_pool(name="small", bufs=4) as spool,
            tc.tile_pool(name="psum", bufs=2, space="PSUM") as pspool,
            tc.tile_pool(name="psum_big", bufs=2, space="PSUM") as psbig,
        ):
            wl_top = cpool.tile([128, H], F32)
            wl_bot = cpool.tile([1, H], F32)
            wr_top = cpool.tile([128, H], F32)
            wr_bot = cpool.tile([1, H], F32)
            nc.sync.dma_start(out=wl_top[:], in_=WlT[0:128, :])
            nc.sync.dma_start(out=wl_bot[:], in_=WlT[128:129, :])
            nc.sync.dma_start(out=wr_top[:], in_=WrT[0:128, :])
            nc.sync.dma_start(out=wr_bot[:], in_=WrT[128:129, :])
            blr = cpool.tile([128, H], F32)
            brr = cpool.tile([128, H], F32)
            ivr = cpool.tile([128, H], F32)
            gbr = cpool.tile([128, H], F32)
            nc.sync.dma_start(out=blr[:], in_=blrep[:, :])
            nc.sync.dma_start(out=brr[:], in_=brrep[:, :])
            nc.sync.dma_start(out=ivr[:], in_=invrep[:, :])
            nc.sync.dma_start(out=gbr[:], in_=gbrep[:, :])
            wo_sb = cpool.tile([H, F], F32)
            nc.sync.dma_start(out=wo_sb[:], in_=WoT[:, :])
            bo_sb = cpool.tile([F, 1], F32)
            nc.sync.dma_start(out=bo_sb[:], in_=bo_col[:, :])
            ident = cpool.tile([128, 128], F32)
            make_identity(nc, ident[:])
            idx_sb = cpool.tile([128, W], I32)
            msk_sb = cpool.tile([128, W], F32)
            nc.sync.dma_start(out=idx_sb[:], in_=idxg[:, :])
            nc.sync.dma_start(out=msk_sb[:], in_=maskb[:, :])

            xltab = dpool.tile([N_tab, H], F32)

            # projection pass A: xl' for all nodes -> DRAM table
            XCHUNK = 16
            n_x_chunks = -(-n_tab_tiles // XCHUNK)
            for xc in range(n_x_chunks):
                t0 = xc * XCHUNK
                t1 = min(t0 + XCHUNK, n_tab_tiles)
                nt = t1 - t0
                c0 = t0 * 128
                cw = min(nt * 128, N - c0)
                xa = xpool.tile([128, XCHUNK * 128], F32, tag="xa")
                xb = xpool.tile([1, XCHUNK * 128], F32, tag="xb")
                nc.sync.dma_start(out=xa[:, 0:cw], in_=xT[0:128, c0:c0 + cw])
                nc.sync.dma_start(out=xb[:, 0:cw], in_=xT[128:129, c0:c0 + cw])
                stage = ppool.tile([128, XCHUNK * H], F32, tag="stage")
                for t in range(t0, t1):
                    j = t - t0
                    ps = pspool.tile([128, H], F32, tag="pp")
                    nc.tensor.matmul(out=ps[:], lhsT=xa[:, j * 128:(j + 1) * 128],
                                     rhs=wl_top[:], start=True, stop=False)
                    nc.tensor.matmul(out=ps[:], lhsT=xb[:, j * 128:(j + 1) * 128],
                                     rhs=wl_bot[:], start=False, stop=True)
                    nc.vector.tensor_tensor(out=stage[:, j * H:(j + 1) * H],
                                            in0=ps[:], in1=blr[:], op=OP.add)
                nc.sync.dma_start(
                    out=xltab[t0 * 128:t1 * 128, :].rearrange(
                        "(t p) h -> p t h", p=128),
                    in_=stage[:, 0:nt * H].rearrange("p (t h) -> p t h", h=H),
                )

            # projection pass B: xr' for own (permuted) nodes -> SBUF
            xr_sb = cpool.tile([128, n_blocks * H], F32)
            n_own_chunks = -(-n_blocks // XCHUNK)
            for xc in range(n_own_chunks):
                t0 = xc * XCHUNK
                t1 = min(t0 + XCHUNK, n_blocks)
                nt = t1 - t0
                c0 = t0 * 128
                xa = xpool.tile([128, XCHUNK * 128], F32, tag="xa")
                xb = xpool.tile([1, XCHUNK * 128], F32, tag="xb")
                nc.sync.dma_start(out=xa[:, 0:nt * 128],
                                  in_=xT_own[0:128, c0:c0 + nt * 128])
                nc.sync.dma_start(out=xb[:, 0:nt * 128],
                                  in_=xT_own[128:129, c0:c0 + nt * 128])
                for t in range(t0, t1):
                    j = t - t0
                    ps = pspool.tile([128, H], F32, tag="pp")
                    nc.tensor.matmul(out=ps[:], lhsT=xa[:, j * 128:(j + 1) * 128],
                                     rhs=wr_top[:], start=True, stop=False)
                    nc.tensor.matmul(out=ps[:], lhsT=xb[:, j * 128:(j + 1) * 128],
                                     rhs=wr_bot[:], start=False, stop=True)
                    nc.vector.tensor_tensor(out=xr_sb[:, t * H:(t + 1) * H],
                                            in0=ps[:], in1=brr[:], op=OP.add)

            # zero gather buffers once (stale-NaN guard; later reuse holds
            # finite values and masked slots contribute exactly 0)
            for _ in range(3):
                gz = gpool.tile([128, Kmax * H], F32, tag="G")
                nc.vector.memset(gz[:], 0.0)

            h_sb = cpool.tile([128, n_blocks * H], F32)
            bounds_reg = nc.gpsimd.to_reg(N - 1)
            for b in range(n_blocks):
                K = Kb[b]
                cs = int(col_off[b])
                G = gpool.tile([128, Kmax * H], F32, tag="G")
                for k in range(K):
                    nc.gpsimd.indirect_dma_start(
                        out=G[:, k * H:(k + 1) * H],
                        out_offset=None,
                        in_=xltab[:, :],
                        in_offset=bass.IndirectOffsetOnAxis(
                            ap=idx_sb[:, cs + k:cs + k + 1], axis=0),
                        bounds_check=bounds_reg,
                        oob_is_err=False,
                    )
                Gv = G[:, 0:K * H].rearrange("p (k h) -> p k h", h=H)
                M = gpool.tile([128, Kmax * H], F32, tag="M")
                Mv = M[:, 0:K * H].rearrange("p (k h) -> p k h", h=H)
                xr_b = xr_sb[:, b * H:(b + 1) * H].rearrange(
                    "p (o h) -> p o h", o=1).to_broadcast([128, K, H])
                nc.vector.tensor_tensor(out=Mv, in0=Gv, in1=xr_b, op=OP.add)
                # leaky-relu slope 0.2 (HW Lrelu table is fixed 0.01):
                # lrelu(x) = 0.2*x + relu(0.8*x)
                R = gpool.tile([128, Kmax * H], F32, tag="R")
                nc.scalar.activation(R[:, 0:K * H], M[:, 0:K * H], AF.Relu,
                                     scale=1.0 - NEG_SLOPE)
                nc.vector.scalar_tensor_tensor(
                    out=M[:, 0:K * H], in0=M[:, 0:K * H], scalar=NEG_SLOPE,
                    op0=OP.mult, in1=R[:, 0:K * H], op1=OP.add)
                ep = spool.tile([128, Kmax], F32, tag="ep")
                en = spool.tile([128, Kmax], F32, tag="en")
                ee = spool.tile([128, Kmax], F32, tag="ee")
                if P > 0:
                    nc.vector.tensor_reduce(ep[:, 0:K], Mv[:, :, 0:P],
                                            axis=AX.X, op=OP.add)
                else:
                    nc.vector.memset(ep[:, 0:K], 0.0)
                if P < H:
                    nc.vector.tensor_reduce(en[:, 0:K], Mv[:, :, P:H],
                                            axis=AX.X, op=OP.add)
                else:
                    nc.vector.memset(en[:, 0:K], 0.0)
                nc.vector.tensor_tensor(out=ee[:, 0:K], in0=ep[:, 0:K],
                                        in1=en[:, 0:K], op=OP.subtract)
                nc.vector.tensor_tensor(out=ee[:, 0:K], in0=ee[:, 0:K],
                                        in1=msk_sb[:, cs:cs + K], op=OP.subtract)
                ex = spool.tile([128, Kmax], F32, tag="ex")
                nc.scalar.activation(ex[:, 0:K], ee[:, 0:K], AF.Exp)
                den = spool.tile([128, 1], F32, tag="den")
                nc.vector.tensor_reduce(den[:], ex[:, 0:K], axis=AX.X, op=OP.add)
                rec = spool.tile([128, 1], F32, tag="rec")
                nc.vector.reciprocal(rec[:], den[:])
                T = gpool.tile([128, Kmax * H], F32, tag="M")
                Tv = T[:, 0:K * H].rearrange("p (k h) -> p k h", h=H)
                exb = ex[:, 0:K].rearrange("p (k o) -> p k o", o=1).to_broadcast(
                    [128, K, H])
                nc.vector.tensor_tensor(out=Tv, in0=Gv, in1=exb, op=OP.mult)
                agg = spool.tile([128, H], F32, tag="agg")
                nc.vector.tensor_reduce(
                    agg[:], T[:, 0:K * H].rearrange("p (k h) -> p h k", h=H),
                    axis=AX.X, op=OP.add)
                ob = spool.tile([128, H], F32, tag="ob")
                nc.scalar.activation(ob[:], agg[:], AF.Copy, scale=rec[:, 0:1])
                nc.vector.tensor_tensor(out=ob[:], in0=ob[:], in1=ivr[:], op=OP.mult)
                nc.vector.tensor_tensor(out=ob[:], in0=ob[:], in1=gbr[:], op=OP.add)
                # elu(x) = relu(x) + exp(min(x,0)) - 1
                xm = spool.tile([128, H], F32, tag="xm")
                nc.vector.tensor_scalar_min(xm[:], ob[:], 0.0)
                em = spool.tile([128, H], F32, tag="em")
                nc.scalar.activation(em[:], xm[:], AF.Exp)
                rl = spool.tile([128, H], F32, tag="rl")
                nc.vector.tensor_tensor(out=rl[:], in0=ob[:], in1=xm[:],
                                        op=OP.subtract)
                nc.vector.scalar_tensor_tensor(
                    out=h_sb[:, b * H:(b + 1) * H], in0=em[:], scalar=-1.0,
                    op0=OP.add, in1=rl[:], op1=OP.add)

            # final logits (transposed): outT = Wo_p @ h.T + bo
            GB = 4
            n_groups = -(-n_blocks // GB)
            for g in range(n_groups):
                b0 = g * GB
                b1 = min(b0 + GB, n_blocks)
                nb = b1 - b0
                hT = ppool.tile([H, GB * 128], F32, tag="hT")
                for b in range(b0, b1):
                    pt = pspool.tile([H, 128], F32, tag="pt")
                    nc.tensor.transpose(out=pt[:], in_=h_sb[:, b * H:(b + 1) * H],
                                        identity=ident[:])
                    nc.vector.tensor_copy(
                        out=hT[:, (b - b0) * 128:(b - b0 + 1) * 128], in_=pt[:])
                po = psbig.tile([F, GB * 128], F32, tag="po")
                nc.tensor.matmul(out=po[:, 0:nb * 128], lhsT=wo_sb[:],
                                 rhs=hT[:, 0:nb * 128], start=True, stop=True)
                ot = ppool.tile([F, GB * 128], F32, tag="ot")
                nc.scalar.activation(ot[:, 0:nb * 128], po[:, 0:nb * 128],
                                     AF.Identity, bias=bo_sb[:, 0:1])
                nc.sync.dma_start(out=outT[:, b0 * 128:b0 * 128 + nb * 128],
                                  in_=ot[:, 0:nb * 128])

    split_excess_waits(nc)
    return nc


# --------------------------------------------------------------------------
# PJRT execution across the 8 cores.
# --------------------------------------------------------------------------

def _run_spmd(nc, per_core, n_cores=N_CORES):
    import jax
    from jax.sharding import Mesh, PartitionSpec
    from jax.experimental.shard_map import shard_map
    from concourse.bass2jax import (_bass_exec_p, partition_id_tensor,
                                    install_neuronx_cc_hook)

    install_neuronx_cc_hook()
    partition_name = nc.partition_id_tensor.name if nc.partition_id_tensor else None
    in_names, out_names, out_avals, zero_outs = [], [], [], []
    for alloc in nc.m.functions[0].allocations:
        if not isinstance(alloc, mybir.MemoryLocationSet):
            continue
        name = alloc.memorylocations[0].name
        if alloc.kind == "ExternalInput":
            if name != partition_name:
                in_names.append(name)
        elif alloc.kind == "ExternalOutput":
            out_names.append(name)
            shape = tuple(alloc.tensor_shape)
            dtype = mybir.dt.np(alloc.dtype)
            out_avals.append(jax.core.ShapedArray(shape, dtype))
            zero_outs.append(np.zeros(shape, dtype))
    n_params = len(in_names)
    n_outs = len(out_avals)
    all_in_names = in_names + out_names + ([partition_name] if partition_name else [])

    def _body(*args):
        operands = list(args)
        if partition_name is not None:
            operands.append(partition_id_tensor())
        outs = _bass_exec_p.bind(
            *operands,
            out_avals=tuple(out_avals),
            in_names=tuple(all_in_names),
            out_names=tuple(out_names),
            lowering_input_output_aliases=(),
            sim_require_finite=True,
            sim_require_nnan=True,
            nc=nc,
        )
        return tuple(outs)

    devices = jax.devices()[:n_cores]
    mesh = Mesh(np.asarray(devices), ("core",))
    in_specs = (PartitionSpec("core"),) * (n_params + n_outs)
    out_specs = (PartitionSpec("core"),) * len(out_names)
    jf = jax.jit(
        shard_map(_body, mesh=mesh, in_specs=in_specs, out_specs=out_specs,
                  check_rep=False),
        keep_unused=True,
    )

    concat_in = [
        np.concatenate([np.asarray(per_core[c][n]) for c in range(n_cores)], axis=0)
        for n in in_names
    ] + [np.concatenate([z] * n_cores, axis=0) for z in zero_outs]
    outs = jf(*concat_in)
    outs = [np.asarray(o) for o in outs]
    res = []
    for c in range(n_cores):
        d = {}
        for i, nm in enumerate(out_names):
            per = outs[i].shape[0] // n_cores
            d[nm] = outs[i][c * per:(c + 1) * per]
        res.append(d)
    return res


_CACHE = {}


def _build(inputs):
    meta, per_core = build_host_data(
        inputs['x_float'], inputs['x_binary'], inputs['edge_index'],
        inputs['Wl'], inputs['bl'], inputs['Wr'], inputs['br'],
        inputs['att'], inputs['gat_bias'], inputs['Wo'], inputs['bo'])
    nc = build_device_program(meta)
    return meta, per_core, nc


def kernel(**inputs):
    inputs = {k: np.asarray(v) for k, v in inputs.items()}
    meta, per_core, nc = _build(inputs)
    res = _run_spmd(nc, per_core)
    N, F = meta['N'], meta['F']
    logits = np.zeros((N, F), dtype=np.float32)
    for c in range(meta['n_cores']):
        nodes = meta['core_nodes'][c]
        oT = res[c]['outT']
        valid = nodes >= 0
        logits[nodes[valid]] = oT[:, valid].T
    disp = np.asarray(inputs['dispersion'], dtype=np.float32)
    softplus = (np.log1p(np.exp(-np.abs(disp))) +
                np.maximum(disp, 0.0)).astype(np.float32)
    return logits, softplus
